# revision 1
# baseline (speedup 1.0000x reference)
"""Bernstein flow density kernel for Trainium2 (8 NeuronCores, data-parallel).

Math (per sample x in R^5, per dim i):
  c = constrained(A_i)                     # [(4)^i, 15] monotone coeffs in (0,1)
  tf_k = sum_j cb_ij c[j,k]                # cb_i = multivariate Bernstein basis over x[:i]
  dcoef_k = tf_k - tf_{k-1}  (tf_{-1}=0, tf_15=1)
  db_k = 16*comb(15,k) x_i^k (1-x_i)^(15-k)
  f_i = sum_k dcoef_k db_k ;  density = prod_i f_i

Device mapping:
  - dcoef directly from matmul: fold the k-difference into the weight matrix
    (column diffs of c; last column = comb - c[:,14] using partition of unity).
  - db via exp(k*ln x + (15-k)*ln u + ln(16 comb)) : one small matmul over
    [ln x; ln u] (fp16 hi+lo split for accuracy) + one ACT Exp.
  - cb built on DVE in fp16 (pure monomials, comb(3,.) folded into weights),
    transposed to basis-major layout with the DMA xbar (2-byte transpose).
  - f_i  = ones-block matmul over dcoef*db rows; density = exp(ones-matmul of ln f).
"""

import math
import sys

import numpy as np

for _p in ("/opt/trn_rl_repo", "/root/.axon_site/_ro/trn_rl_repo"):
    if _p not in sys.path:
        sys.path.append(_p)

import concourse.bass as bass
import concourse.tile as tile
from concourse import bacc, mybir
from concourse.bass_utils import run_bass_kernel_spmd

F32 = mybir.dt.float32
F16 = mybir.dt.float16
F32R = mybir.dt.float32r

DIM = 5
TF_DEG = 16
N_FULL = 262144
N_CORES = 8
N_CORE = N_FULL // N_CORES  # 32768
SC = 256.0  # scale folded into dcoef weights to keep fp16 away from subnormals
COMB3 = np.array([1.0, 3.0, 3.0, 1.0])
COMB15 = np.array([math.comb(15, k) for k in range(16)], dtype=np.float64)


# ----------------------------------------------------------------- host consts
def _constrained(A):
    A = A.astype(np.float64)
    sp = np.log1p(np.exp(-np.abs(A))) + np.maximum(A, 0.0)  # softplus, stable
    cs = np.cumsum(sp, axis=1)
    return 2.0 * (1.0 / (1.0 + np.exp(-cs)) - 0.5)


def _dev_perm_scale(i):
    """Map device row p (j_{i-1} slowest ... j_0 fastest reversed: p = sum_d j_d*4^d)
    to reference row (j_0 slowest: ref = sum_d j_d*4^(i-1-d)) + comb scale."""
    rows = 4**i
    ref_idx = np.zeros(rows, dtype=np.int64)
    scale = np.ones(rows)
    for p in range(rows):
        r = 0
        s = 1.0
        for d in range(i):  # j_d = digit d of p (j_0 = fastest)
            jd = (p >> (2 * d)) & 3
            r += jd * 4 ** (i - 1 - d)
            s *= COMB3[jd]
        ref_idx[p] = r
        scale[p] = s
    return ref_idx, scale


def _dcoef_weights(C, combscale):
    """C: [rows,15] device-row-ORDERED original coeffs (not comb-divided);
    returns [rows,16] W with the tf-difference folded in, scaled so that
    sum_j monomial_j * W[j,k] = SC * dcoef_k."""
    rows = C.shape[0]
    W = np.zeros((rows, 16))
    W[:, 0] = C[:, 0]
    W[:, 1:15] = C[:, 1:15] - C[:, 0:14]
    W[:, 15] = 1.0 - C[:, 14]
    return W * combscale[:, None] * SC


def build_consts(A_list):
    Cs = []
    for i in range(DIM):
        C = _constrained(A_list[i])  # [(4)^i, 15] in reference row order
        if i == 0:
            Cs.append((C, np.ones(1)))
        else:
            ref_idx, scale = _dev_perm_scale(i)
            # device row p uses reference row ref_idx[p]; comb folded via scale
            Cs.append((C[ref_idx], scale))
    Wd = [_dcoef_weights(Cperm, scale) for (Cperm, scale) in Cs]  # [rows,16] each

    # fp16 dynamic range fix: scale each (dim>=1, k) weight column by a power
    # of two so its max lands near 1024 (away from fp16 subnormals), and fold
    # the inverse into the exp bias of the matching db row (exact compensation).
    colshift = np.zeros((5, 16))  # ln of the applied scale, for expbias
    for i in range(5):
        m = np.max(np.abs(Wd[i]), axis=0)  # [16]
        e = np.round(np.log2(1024.0 / np.maximum(m, 1e-300)))
        e = np.clip(e, -10, 40)
        s = np.exp2(e)
        Wd[i] = Wd[i] * s[None, :]
        colshift[i] = e * math.log(2.0)

    w13 = np.zeros((84, 64))
    w13[0:4, 0:16] = Wd[1]
    w13[4:20, 16:32] = Wd[2]
    w13[20:84, 32:48] = Wd[3]
    w4a = Wd[4][0:128]  # [128,16]
    w4b = Wd[4][128:256]

    # Wlog: rhs rows q (0:20) -> [lnx_d hi (0:5), lnu_d hi (5:10), lnx lo, lnu lo]
    # out rows r (0:96): 0:48 d=1..3 k=0:16 ; 48:64 zero ; 64:80 d=4 ; 80:96 d=0
    wlog = np.zeros((20, 96))

    def _rowmap(r):
        if r < 48:
            return 1 + r // 16, r % 16
        if r < 64:
            return None
        if r < 80:
            return 4, r - 64
        return 0, r - 80

    for r in range(96):
        mk = _rowmap(r)
        if mk is None:
            continue
        d, k = mk
        for base in (0, 10):  # hi rows and lo rows share coefficients
            wlog[base + d, r] = float(k)
            wlog[base + 5 + d, r] = float(15 - k)
    # single band-masked copy: ln values ride inside the cbA region at rows
    # 84:104 after the block transpose; K=128 matmul with zeros elsewhere
    wlog1 = np.zeros((128, 96))
    wlog1[84:104, :] = wlog

    expbias = np.zeros((96, 1))
    for r in range(96):
        mk = _rowmap(r)
        if mk is None:
            continue
        d, k = mk
        expbias[r, 0] = math.log(16.0 * COMB15[k]) - colshift[d, k]

    f1w = np.zeros((80, 32))
    for i in (1, 2, 3):
        f1w[(i - 1) * 16 : i * 16, i] = 1.0
    f1w[64:80, 4] = 1.0

    f2w = np.zeros((96, 32))
    f2w[80:96, 0] = Wd[0][0]  # dcoef weights of dim 0 applied to db d=0 rows
    f2w[48, 5:32] = 1.0  # pad cols read dbT row 48 (==1.0): psum pad stays ln-safe

    lnones = np.zeros((128, 4))
    for t in range(4):
        lnones[32 * t : 32 * t + 5, t] = 1.0

    fbias = np.full((4, 1), -DIM * math.log(SC))

    return {
        "fbias": fbias.astype(np.float32),
        "w13": w13.astype(np.float16),
        "w4a": w4a.astype(np.float16),
        "w4b": w4b.astype(np.float16),
        "wlog1": wlog1.astype(np.float16),
        "expbias": expbias.astype(np.float32),
        "f1w": f1w.astype(np.float16),
        "f2w": f2w.astype(np.float16),
        "lnones": lnones.astype(np.float32),
    }


# ---------------------------------------------------------------- device build
def _ap(t, extra_offset, dims):
    """Manual AP over a tile: keep its partition dim, custom free dims."""
    return bass.AP(
        tensor=t.tensor, offset=t.offset + extra_offset, ap=[list(t.ap[0])] + dims
    )


def build_nc(ncore, nblk):
    """nblk = sub-tiles (128 samples each) per block; must be mult of 16."""
    assert nblk % 16 == 0
    nsamp_blk = 128 * nblk
    assert ncore % nsamp_blk == 0
    nblocks = ncore // nsamp_blk
    ngroups = nblk // 4  # 512-sample groups per block
    xcols = ncore // 128 * DIM

    nc = bacc.Bacc("TRN2", target_bir_lowering=False, debug=False, num_devices=N_CORES)
    xt = nc.declare_dram_parameter("xt", [128, xcols], F32, isOutput=False)
    w13 = nc.declare_dram_parameter("w13", [84, 64], F16, isOutput=False)
    w4a = nc.declare_dram_parameter("w4a", [128, 16], F16, isOutput=False)
    w4b = nc.declare_dram_parameter("w4b", [128, 16], F16, isOutput=False)
    wlog1 = nc.declare_dram_parameter("wlog1", [128, 96], F16, isOutput=False)
    expbias = nc.declare_dram_parameter("expbias", [96, 1], F32, isOutput=False)
    f1w = nc.declare_dram_parameter("f1w", [80, 32], F16, isOutput=False)
    f2w = nc.declare_dram_parameter("f2w", [96, 32], F16, isOutput=False)
    lnones = nc.declare_dram_parameter("lnones", [128, 4], F32, isOutput=False)
    fbias = nc.declare_dram_parameter("fbias", [4, 1], F32, isOutput=False)
    dens = nc.declare_dram_parameter("dens", [ncore], F32, isOutput=True)

    Exp = mybir.ActivationFunctionType.Exp
    Ln = mybir.ActivationFunctionType.Ln

    with tile.TileContext(nc) as tc:
        with (
            tc.tile_pool(name="wc", bufs=1) as wc,
            tc.tile_pool(name="la", bufs=2) as la,
            tc.tile_pool(name="gr", bufs=3) as gr,
            tc.tile_pool(name="tr", bufs=2) as tr,
            tc.tile_pool(name="sb", bufs=2) as sbp,
            tc.tile_pool(name="psg", bufs=2, space="PSUM") as psg,
            tc.tile_pool(name="psf", bufs=2, space="PSUM") as psf,
            tc.tile_pool(name="psd", bufs=2, space="PSUM") as psd,
        ):
            w13sb = wc.tile([84, 64], F16, tag="w13")
            w4asb = wc.tile([128, 16], F16, tag="w4a")
            w4bsb = wc.tile([128, 16], F16, tag="w4b")
            wlogsb = wc.tile([128, 96], F16, tag="wlog")
            expbsb = wc.tile([96, 1], F32, tag="expb")
            f1wsb = wc.tile([80, 32], F16, tag="f1w")
            f2wsb = wc.tile([96, 32], F16, tag="f2w")
            lnosb = wc.tile([128, 4], F32, tag="lno")
            fbsb = wc.tile([4, 1], F32, tag="fb")
            xall = wc.tile([128, xcols], F32, tag="xall")
            for dst, src in (
                (w13sb, w13),
                (w4asb, w4a),
                (w4bsb, w4b),
                (wlogsb, wlog1),
                (expbsb, expbias),
                (f1wsb, f1w),
                (f2wsb, f2w),
                (lnosb, lnones),
                (fbsb, fbias),
                (xall, xt),
            ):
                nc.gpsimd.dma_start(out=dst[:], in_=src[:])

            for blk in range(nblocks):
                n = nblk
                xa = xall[:, blk * n * 5 : (blk + 1) * n * 5].rearrange(
                    "p (n d) -> p n d", d=5
                )
                u = la.tile([128, n, 4], F32, tag="u")
                xp2 = la.tile([128, n, 4], F32, tag="xp2")
                up2 = la.tile([128, n, 4], F32, tag="up2")
                ln32 = la.tile([128, n, 10], F32, tag="ln32")
                b4 = la.tile([128, n, 4, 4], F16, tag="b4")
                # per-sub-tile 128-col regions (contiguous -> one block transpose each)
                # cbA: 0:4 cb1 | 4:20 cb2 | 20:84 cb3 | 84:104 ln hi/lo | 104:128 zero
                cbA = la.tile([128, n, 128], F16, tag="cbA")
                cbB = la.tile([128, n, 128], F16, tag="cbB")  # cb4 rows 0:128
                cbC = la.tile([128, n, 128], F16, tag="cbC")  # cb4 rows 128:256

                x4 = xa[:, :, 0:4]
                nc.vector.tensor_scalar(
                    out=u[:],
                    in0=x4,
                    scalar1=1.0,
                    scalar2=-1.0,
                    op0=mybir.AluOpType.subtract,
                    op1=mybir.AluOpType.mult,
                )
                nc.vector.tensor_mul(out=xp2[:], in0=x4, in1=x4)
                nc.vector.tensor_mul(out=up2[:], in0=u[:], in1=u[:])
                nc.scalar.activation(out=ln32[:, :, 0:5], in_=xa, func=Ln)
                nc.scalar.activation(
                    out=ln32[:, :, 5:10], in_=xa, func=Ln, scale=-1.0, bias=1.0
                )
                nc.vector.tensor_copy(out=cbA[:, :, 84:94], in_=ln32[:])
                nc.vector.tensor_sub(
                    out=cbA[:, :, 94:104], in0=ln32[:], in1=cbA[:, :, 84:94]
                )
                nc.vector.memset(cbA[:, :, 104:128], 0.0)
                # b4[:, :, j, d]: j0=u^3, j1=x u^2, j2=x^2 u, j3=x^3 (d=0..3)
                nc.vector.tensor_mul(out=b4[:, :, 0, :], in0=up2[:], in1=u[:])
                nc.vector.tensor_mul(out=b4[:, :, 1, :], in0=x4, in1=up2[:])
                nc.vector.tensor_mul(out=b4[:, :, 2, :], in0=xp2[:], in1=u[:])
                nc.vector.tensor_mul(out=b4[:, :, 3, :], in0=xp2[:], in1=x4)
                nc.vector.tensor_copy(
                    out=cbA[:, :, 0:4], in_=_ap(b4[:], 0, [[16, n], [4, 4]])
                )
                nc.vector.tensor_mul(
                    out=cbA[:, :, 4:20].rearrange("p n (a b) -> p n a b", a=4),
                    in0=_ap(b4[:], 0, [[16, n], [0, 4], [4, 4]]),
                    in1=_ap(b4[:], 1, [[16, n], [4, 4], [0, 4]]),
                )
                nc.vector.tensor_mul(
                    out=cbA[:, :, 20:84].rearrange("p n (a b) -> p n a b", a=4),
                    in0=_ap(cbA[:], 4, [[128, n], [0, 4], [1, 16]]),
                    in1=_ap(b4[:], 2, [[16, n], [4, 4], [0, 16]]),
                )
                nc.vector.tensor_mul(
                    out=cbB[:].rearrange("p n (a b) -> p n a b", a=2),
                    in0=_ap(cbA[:], 20, [[128, n], [0, 2], [1, 64]]),
                    in1=_ap(b4[:], 3, [[16, n], [4, 2], [0, 64]]),
                )
                nc.gpsimd.tensor_mul(
                    out=cbC[:].rearrange("p n (a b) -> p n a b", a=2),
                    in0=_ap(cbA[:], 20, [[128, n], [0, 2], [1, 64]]),
                    in1=_ap(b4[:], 11, [[16, n], [4, 2], [0, 64]]),
                )

                # one batched xbar transpose per region per block:
                # out[:, j, :] = transpose(in[:, j*128:(j+1)*128])
                cbTA = tr.tile([128, n, 128], F16, tag="cbTA")
                cbTB = tr.tile([128, n, 128], F16, tag="cbTB")
                cbTC = tr.tile([128, n, 128], F16, tag="cbTC")
                for dst_t, src_t in ((cbTA, cbA), (cbTB, cbB), (cbTC, cbC)):
                    nc.sync.dma_start(
                        out=dst_t[:],
                        in_=src_t[:].rearrange("p n c -> p (n c)"),
                        transpose=True,
                    )

                for sb in range(ngroups // 4):  # superblock: 4 groups / 2048 samples
                    fpsum = psf.tile([128, 512], F32, tag="fpsum")
                    for tp in range(4):
                        g = sb * 4 + tp
                        gsl = slice(4 * g, 4 * g + 4)
                        wlogp = psg.tile([96, 512], F32, tag="wlogp")
                        for t in range(4):
                            nc.tensor.matmul(
                                out=wlogp[:, t * 128 : (t + 1) * 128],
                                lhsT=wlogsb[:],
                                rhs=cbTA[:, 4 * g + t, :],
                                start=True,
                                stop=True,
                            )
                        dbT = gr.tile([96, 512], F16, tag="dbT")
                        nc.scalar.activation(
                            out=dbT[:], in_=wlogp[:], func=Exp, bias=expbsb[:]
                        )
                        dtfp = psg.tile([96, 512], F32, tag="dtfp")
                        nc.tensor.matmul(
                            out=dtfp[0:64, :],
                            lhsT=w13sb[:],
                            rhs=cbTA[0:84, gsl, :],
                            start=True,
                            stop=True,
                        )
                        nc.tensor.matmul(
                            out=dtfp[64:80, :],
                            lhsT=w4asb[:],
                            rhs=cbTB[:, gsl, :],
                            start=True,
                            stop=False,
                        )
                        nc.tensor.matmul(
                            out=dtfp[64:80, :],
                            lhsT=w4bsb[:],
                            rhs=cbTC[:, gsl, :],
                            start=False,
                            stop=True,
                        )
                        prod = gr.tile([80, 512], F16, tag="prod")
                        nc.vector.tensor_mul(
                            out=prod[:], in0=dtfp[0:80, :], in1=dbT[0:80, :]
                        )
                        frows = slice(32 * tp, 32 * tp + 32)
                        nc.tensor.matmul(
                            out=fpsum[frows, :],
                            lhsT=f1wsb[:],
                            rhs=prod[:],
                            start=True,
                            stop=False,
                            tile_position=(0, 32 * tp),
                        )
                        nc.tensor.matmul(
                            out=fpsum[frows, :],
                            lhsT=f2wsb[:],
                            rhs=dbT[:],
                            start=False,
                            stop=True,
                            tile_position=(0, 32 * tp),
                        )
                    lnf = sbp.tile([128, 512], F32, tag="lnf")
                    nc.scalar.activation(out=lnf[:], in_=fpsum[:], func=Ln)
                    lnden = psd.tile([4, 512], F32, tag="lnden")
                    nc.tensor.matmul(
                        out=lnden[:],
                        lhsT=lnosb[:],
                        rhs=lnf[:],
                        start=True,
                        stop=True,
                    )
                    dens_sb = sbp.tile([4, 512], F32, tag="dens_sb")
                    nc.scalar.activation(
                        out=dens_sb[:],
                        in_=lnden[:],
                        func=Exp,
                        bias=fbsb[:],
                    )
                    base = blk * nsamp_blk + sb * 2048
                    nc.gpsimd.dma_start(
                        out=dens[base : base + 2048].rearrange("(t s) -> t s", t=4),
                        in_=dens_sb[:],
                    )
    nc.finalize()
    return nc


# -------------------------------------------------------------------- host run
def pack_x(x_shard):
    """[N_CORE, 5] -> [128, N_CORE/128*5]; sample s = nb*128+p -> row p, cols nb*5+d."""
    n = x_shard.shape[0]
    return (
        np.ascontiguousarray(x_shard.reshape(n // 128, 128, 5).transpose(1, 0, 2))
        .reshape(128, n // 128 * 5)
        .astype(np.float32)
    )


_CACHE = {}


def _get_runner():
    """Build nc + a cached jitted shard_map callable (trace/compile once)."""
    if "runner" in _CACHE:
        return _CACHE["runner"]
    import jax
    from jax.sharding import Mesh, PartitionSpec
    from jax.experimental.shard_map import shard_map

    from concourse import bass2jax, mybir as _mb
    from concourse.bass2jax import (
        _bass_exec_p,
        install_neuronx_cc_hook,
        partition_id_tensor,
    )

    install_neuronx_cc_hook()
    nc = build_nc(N_CORE, 32)
    partition_name = nc.partition_id_tensor.name if nc.partition_id_tensor else None

    in_names, out_names, out_avals, zero_outs = [], [], [], []
    for alloc in nc.m.functions[0].allocations:
        if not isinstance(alloc, _mb.MemoryLocationSet):
            continue
        name = alloc.memorylocations[0].name
        if alloc.kind == "ExternalInput":
            if name != partition_name:
                in_names.append(name)
        elif alloc.kind == "ExternalOutput":
            out_names.append(name)
            shape = tuple(alloc.tensor_shape)
            dtype = _mb.dt.np(alloc.dtype)
            out_avals.append(jax.core.ShapedArray(shape, dtype))
            zero_outs.append(np.zeros(shape, dtype))
    n_params = len(in_names)
    all_in_names = list(in_names) + list(out_names)
    if partition_name is not None:
        all_in_names.append(partition_name)

    def _body(*args):
        operands = list(args)
        if partition_name is not None:
            operands.append(partition_id_tensor())
        outs = _bass_exec_p.bind(
            *operands,
            out_avals=tuple(out_avals),
            in_names=tuple(all_in_names),
            out_names=tuple(out_names),
            lowering_input_output_aliases=(),
            sim_require_finite=True,
            sim_require_nnan=True,
            nc=nc,
        )
        return tuple(outs)

    devices = jax.devices()[:N_CORES]
    mesh = Mesh(np.asarray(devices), ("core",))
    in_specs = (PartitionSpec("core"),) * (n_params + len(out_names))
    out_specs = (PartitionSpec("core"),) * len(out_names)
    sharded = jax.jit(
        shard_map(
            _body, mesh=mesh, in_specs=in_specs, out_specs=out_specs, check_rep=False
        ),
        keep_unused=True,
    )
    shard = jax.NamedSharding(mesh, PartitionSpec("core"))
    zeros_dev = [
        jax.device_put(
            np.zeros((N_CORES * z.shape[0], *z.shape[1:]), z.dtype), shard
        )
        for z in zero_outs
    ]
    _CACHE["runner"] = (sharded, in_names, out_names, out_avals, zeros_dev, shard)
    return _CACHE["runner"]


def run_device(in_maps):
    """in_maps: per-core dicts. Returns list of per-core output dicts."""
    import jax

    sharded, in_names, out_names, out_avals, zeros_dev, shard = _get_runner()
    concat_in = [
        jax.device_put(
            np.concatenate(
                [np.asarray(in_maps[c][k]) for c in range(N_CORES)], axis=0
            ),
            shard,
        )
        for k in in_names
    ]
    out_arrs = sharded(*concat_in, *zeros_dev)
    return [
        {
            k: np.asarray(out_arrs[i]).reshape(N_CORES, *out_avals[i].shape)[c]
            for i, k in enumerate(out_names)
        }
        for c in range(N_CORES)
    ]


def make_in_maps(x, A_list):
    consts = build_consts([np.asarray(a) for a in A_list])
    in_maps = []
    for c in range(N_CORES):
        m = {"xt": pack_x(x[c * N_CORE : (c + 1) * N_CORE])}
        m.update(consts)
        in_maps.append(m)
    return in_maps


def kernel(x, A0, A1, A2, A3, A4):
    x = np.asarray(x, dtype=np.float32)
    in_maps = make_in_maps(x, (A0, A1, A2, A3, A4))
    res = run_device(in_maps)
    return np.concatenate([res[c]["dens"] for c in range(N_CORES)])

